# revision 2
# baseline (speedup 1.0000x reference)
"""Trainium2 Bass kernel for nn_MessagePassingLayer (GNN message passing), v5.

Like v4 (host LPT cell packing ~4% padding, pair-packed [128, S/2] bf16
stream, host-built fp8 one-hot scatter blocks, W2 folded into gate/update
weights, node-major phase 3, software-pipelined PE stream, batched-rstd
LayerNorm tail) but with one crucial hardware fix:

Matmuls whose operands sit at partition base 64 crash the runtime when
interleaved with base-0 matmuls (PE quadrant switching). So every matmul
here contracts over all 128 partitions with BLOCK-DIAGONAL weights:
- stream pairs: h for chunks (2P, 2P+1) = stream_block[128, 128] @
  blockdiag(W1, W1) -> [128 slots, 128] (two 64-col h halves).
- phase 3 pairs windows (w, w+2) on partition halves of the packed
  aggregate/nfT tiles and uses blockdiag(Wgp), blockdiag(Wgb), etc.
Window slots are emitted in memory order [w0, w2, w1, w3] per group; the
host unscrambles via the column map. X/Xo carry 4*ngroups slots (100).
"""
import heapq
import math
import os
import numpy as np
import ml_dtypes

import concourse.bass as bass
import concourse.bacc as bacc
import concourse.tile as tile
from concourse import mybir
from concourse.bass_utils import run_bass_kernel_spmd

BF = ml_dtypes.bfloat16
F8 = ml_dtypes.float8_e4m3
N = 100000
D = 64
E = 1250000
NCORES = 8
NPC = 12544
WPC = 98
SUBW = 16
NCELL = NPC // SUBW
LN_EPS = 1e-3
GRP = 4
CPG = GRP * 128 // SUBW  # 32

_CACHE = {}


def _pack_cells(deg):
    order = np.argsort(-deg, kind="stable")
    heap = [(0, 0, c) for c in range(NCELL)]
    heapq.heapify(heap)
    cell_id = np.empty(NPC, np.int32)
    pos = np.empty(NPC, np.int32)
    for i in order:
        s, cnt, c = heapq.heappop(heap)
        cell_id[i] = c
        pos[i] = cnt
        if cnt + 1 < SUBW:
            heapq.heappush(heap, (s + int(deg[i]), cnt + 1, c))
    return cell_id, pos


def _host_prep(node_feats, neighbor_feats, edge_indices, edge_weights):
    src = np.asarray(edge_indices[0], dtype=np.int64)
    dst = np.asarray(edge_indices[1], dtype=np.int64)
    w = np.asarray(edge_weights[:, 0], dtype=np.float64)

    wdeg = np.bincount(dst, weights=w, minlength=N)
    recdeg = 1.0 / np.maximum(wdeg, 1.0)
    wprime = (w * recdeg[dst]).astype(np.float32)
    sumw = (wdeg * recdeg).astype(np.float32)

    cnt = np.zeros(NCORES * NPC, np.int64)
    cnt[:N] = np.bincount(dst, minlength=N)

    cell_id = np.empty(NCORES * NPC, np.int32)
    pos_in = np.empty(NCORES * NPC, np.int32)
    maxcell = 0
    for k in range(NCORES):
        lo = k * NPC
        cid, p = _pack_cells(cnt[lo:lo + NPC])
        cell_id[lo:lo + NPC] = cid
        pos_in[lo:lo + NPC] = p
        sums = np.bincount(cid, weights=cnt[lo:lo + NPC], minlength=NCELL)
        maxcell = max(maxcell, int(sums.max()))
    L = max(16 * math.ceil(maxcell / 16), 144)
    nch = NCELL * L // 128
    if nch % 2:
        nch += 1
    S = nch * 128
    ngroups = math.ceil(WPC / GRP)
    nslot = ngroups * GRP          # 100 window slots (2 padding)

    core = dst // NPC
    ecell = cell_id[dst]
    epos = pos_in[dst]

    gcell = core.astype(np.int64) * NCELL + ecell
    order = np.argsort(gcell, kind="stable")
    gc_s = gcell[order]
    counts = np.bincount(gc_s, minlength=NCORES * NCELL)
    starts = np.zeros(NCORES * NCELL + 1, np.int64)
    starts[1:] = np.cumsum(counts)
    rank = np.arange(E) - starts[gc_s]
    slot = (gc_s % NCELL) * L + rank
    src_s = src[order]
    pos_s = epos[order]
    wp_s = wprime[order]
    core_s = core[order]

    nfeats = np.asarray(neighbor_feats, dtype=np.float32)
    node_f = np.asarray(node_feats, dtype=np.float32)

    # cell c (global, per core) geometry:
    #   group g = c // CPG, ci = c % CPG, window wi = ci // 8 (0..3),
    #   j = ci % 8. Memory: partition half = 64*(ci//16) [= wi//2],
    #   node-col within group block = (wi%2)*128 + j*16 + pos.
    # window slot order per group: q(wi) = (wi%2)*2 + wi//2  ([0,2,1,3]).
    cells = np.arange(NCELL)
    cg = cells // CPG
    ci = cells % CPG
    wi = ci // 8
    jj = ci % 8
    qslot = cg * GRP + (wi % 2) * 2 + (wi // 2)     # memory window slot
    half = (wi // 2)                                 # 0 or 1 (partition)
    # node-major output position: Xo[p, qslot, :] rows p = node col in win
    # node col within window = j*16 + pos
    # colmap entry for node: qslot*128 + j*16 + pos
    cell_q = qslot
    cell_j = jj
    colmap_cell = cell_q * 128 + cell_j * SUBW       # + pos

    # per-cell aggregation PSUM position within group tile [128, 256]:
    #   partitions 64*half + feat, cols (wi%2)*128 + j*16
    agg_pb = 64 * half
    agg_col = (wi % 2) * 128 + jj * SUBW

    per_core = []
    for k in range(NCORES):
        m = core_s == k
        sl = slot[m]
        nfg = np.zeros((S, D), np.float32)
        nfg[sl] = nfeats[src_s[m]] * wp_s[m][:, None]
        nfgP = np.ascontiguousarray(
            nfg.reshape(nch // 2, 2, 128, D).transpose(1, 3, 0, 2)
            .reshape(128, S // 2))

        chunk_of = sl // 128
        first_cell = (chunk_of * 128) // L
        ecl = sl // L
        mcol = (pos_s[m] + SUBW * (ecl - first_cell)).astype(np.int64)
        M = np.zeros((nch, 128, 2 * SUBW), np.float32)
        M[chunk_of, sl % 128, mcol] = 1.0
        M32 = np.ascontiguousarray(M.transpose(1, 0, 2)).reshape(
            128, nch * 2 * SUBW)

        wr = np.zeros((1, S), np.float32)
        wr[0, sl] = wp_s[m]

        lo = k * NPC
        hi = min(lo + NPC, N)
        nfp = np.zeros((NPC, D), np.float32)
        nfp[: hi - lo] = node_f[lo:hi]
        nl = np.arange(hi - lo)
        cell_n = cell_id[lo:lo + NPC][: hi - lo]
        pos_n = pos_in[lo:lo + NPC][: hi - lo]
        # packed feature-major nfT [128, ngroups*256]:
        #   partition 64*half + feat, col g*256 + (wi%2)*128 + j*16 + pos
        g_n = cell_n // CPG
        ci_n = cell_n % CPG
        wi_n = ci_n // 8
        j_n = ci_n % 8
        half_n = wi_n // 2
        colT = g_n * 256 + (wi_n % 2) * 128 + j_n * SUBW + pos_n
        ngroups_ = ngroups
        nfT = np.zeros((2 * D, ngroups_ * 256), np.float32)
        nfT[(half_n[None, :] * D + np.arange(D)[:, None]),
            colT[None, :]] = nfp[: hi - lo].T
        # node-major nfNM [128, nslot*64]: row = node col in window,
        # col = qslot*64 + feat
        q_n = g_n * GRP + (wi_n % 2) * 2 + half_n
        row_n = j_n * SUBW + pos_n
        nfNM = np.zeros((128, nslot * D), np.float32)
        nfNM[row_n[:, None],
             (q_n * D)[:, None] + np.arange(D)[None, :]] = nfp[: hi - lo]
        swv = np.zeros((128, nslot), np.float32)
        swv[row_n, q_n] = sumw[lo:hi][: hi - lo]
        cm = (q_n * 128 + row_n)
        per_core.append({
            "nfgP": nfgP.astype(BF),
            "M32": M32.astype(F8),
            "nfT": nfT.astype(BF),
            "nfNM": nfNM.astype(BF),
            "_sumwNM": swv.astype(np.float32),   # [128, nslot] if b2 != 0
            "_wrow": wr.astype(BF),
            "_colmap": cm,
        })
    return per_core, L, nch


def _const_inputs(W1, b1, W2, b2, Wg, bg, Wu1, bu1, Wu2, bu2, gamma, beta):
    flags = {
        "has_b1": bool(np.any(np.asarray(b1) != 0)),
        "has_b2": bool(np.any(np.asarray(b2) != 0)),
        "has_bg": bool(np.any(np.asarray(bg) != 0)),
        "has_bu2": bool(np.any(np.asarray(bu2) != 0)),
        "has_gb": bool(np.any(np.asarray(gamma) != 1)
                       or np.any(np.asarray(beta) != 0)),
    }
    W2f = np.asarray(W2, np.float64)
    Wgf = np.asarray(Wg, np.float64)
    Wu1f = np.asarray(Wu1, np.float64)

    def blk(a):
        a = np.asarray(a, np.float32)
        z = np.zeros_like(a)
        return np.block([[a, z], [z, a]]).astype(BF)   # [128, 128]

    c = {
        "W1B": blk(W1),
        "WgpB": blk((W2f @ Wgf[:D]).astype(np.float32)),
        "WgbB": blk(Wgf[D:].astype(np.float32)),
        "Wu1pB": blk((W2f @ Wu1f[:D]).astype(np.float32)),
        "Wu1bB": blk(Wu1f[D:].astype(np.float32)),
        "Wu2B": blk(Wu2),
        "bu1": np.tile(np.asarray(bu1, np.float32), 2)[:, None],
    }
    # general-bias fallbacks (zero in this model): materialized lane tiles
    if flags["has_bg"]:
        bgp = np.tile(np.asarray(bg, np.float32), 2)  # gpre block layout
        c["bgT"] = np.tile(bgp[None, :], (128, 1))    # [128, 128]
    if flags["has_bu2"]:
        b2p = np.tile(np.asarray(bu2, np.float32), 2)
        c["bu2T"] = np.tile(b2p[None, :], (128, 1))
    if flags["has_b2"]:
        c["b2gRow"] = np.tile((np.asarray(b2, np.float64) @ Wgf[:D])
                              .astype(np.float32), 2)[None, :]  # [1,128]
        c["b2uRow"] = np.tile((np.asarray(b2, np.float64) @ Wu1f[:D])
                              .astype(np.float32), 2)[None, :]
    if flags["has_b1"]:
        c["b1T"] = np.tile(np.asarray(b1, np.float32)[None, :], (128, 8))
    if flags["has_gb"]:
        c["gamma"] = np.tile(np.asarray(gamma, np.float32)[None, :], (128, 1))
        c["beta"] = np.tile(np.asarray(beta, np.float32)[None, :], (128, 1))
    return c, flags


def _relu_pattern(n, wa, wd):
    out = []
    acc_a = acc_d = 0.0
    for _ in range(n):
        acc_a += wa
        acc_d += wd
        if acc_a >= acc_d:
            acc_a -= wa + wd
            out.append("a")
        else:
            acc_d -= wa + wd
            out.append("d")
    return out


def _build_nc(L, nch, flags):
    RELUB = int(os.environ.get("GNN_RELUB", "8"))
    DMACH = int(os.environ.get("GNN_DMACH", "32"))
    SBUFS = int(os.environ.get("GNN_SBUFS", "5"))
    QPAT = os.environ.get("GNN_QPAT", "s")
    WA = int(os.environ.get("GNN_WA", "100"))
    WD = int(os.environ.get("GNN_WD", "100"))
    HBUFS = int(os.environ.get("GNN_HBUFS", "3"))
    ABUFS = int(os.environ.get("GNN_ABUFS", "1"))
    LOOK = int(os.environ.get("GNN_LOOK", "4"))
    SB = int(os.environ.get("GNN_SB", "3"))
    SC = int(os.environ.get("GNN_SC", "7"))
    MGRPS = int(os.environ.get("GNN_MGRPS", "4"))
    QLOAD = os.environ.get("GNN_QLOAD", "p")
    bf16, f32 = mybir.dt.bfloat16, mybir.dt.float32
    fp8 = mybir.dt.float8e4
    S = nch * 128
    assert nch % 2 == 0 and DMACH % 2 == 0 and RELUB % 2 == 0
    nc = bacc.Bacc("TRN2", target_bir_lowering=False, debug=False)

    ngroups = math.ceil(WPC / GRP)
    nslot = ngroups * GRP

    t_nfgP = nc.dram_tensor("nfgP", (128, S // 2), bf16, kind="ExternalInput")
    t_M32 = nc.dram_tensor("M32", (128, nch * 2 * SUBW), fp8,
                           kind="ExternalInput")
    t_nfT = nc.dram_tensor("nfT", (2 * D, ngroups * 256), bf16,
                           kind="ExternalInput")
    t_nfNM = nc.dram_tensor("nfNM", (128, nslot * D), bf16,
                            kind="ExternalInput")
    t_W1B = nc.dram_tensor("W1B", (128, 128), bf16, kind="ExternalInput")
    t_WgpB = nc.dram_tensor("WgpB", (128, 128), bf16, kind="ExternalInput")
    t_WgbB = nc.dram_tensor("WgbB", (128, 128), bf16, kind="ExternalInput")
    t_Wu1pB = nc.dram_tensor("Wu1pB", (128, 128), bf16, kind="ExternalInput")
    t_Wu1bB = nc.dram_tensor("Wu1bB", (128, 128), bf16, kind="ExternalInput")
    t_Wu2B = nc.dram_tensor("Wu2B", (128, 128), bf16, kind="ExternalInput")
    t_bu1 = nc.dram_tensor("bu1", (2 * D, 1), f32, kind="ExternalInput")
    if flags["has_b1"]:
        t_b1T = nc.dram_tensor("b1T", (128, RELUB * D), f32,
                               kind="ExternalInput")
        t_wrow = nc.dram_tensor("wrow", (1, S), bf16, kind="ExternalInput")
    if flags["has_b2"]:
        t_b2gRow = nc.dram_tensor("b2gRow", (1, 128), f32,
                                  kind="ExternalInput")
        t_b2uRow = nc.dram_tensor("b2uRow", (1, 128), f32,
                                  kind="ExternalInput")
        t_sumwNM = nc.dram_tensor("sumwNM", (128, nslot), f32,
                                  kind="ExternalInput")
    if flags["has_bg"]:
        t_bgT = nc.dram_tensor("bgT", (128, 128), f32, kind="ExternalInput")
    if flags["has_bu2"]:
        t_bu2T = nc.dram_tensor("bu2T", (128, 128), f32, kind="ExternalInput")
    if flags["has_gb"]:
        t_gamma = nc.dram_tensor("gamma", (128, D), f32, kind="ExternalInput")
        t_beta = nc.dram_tensor("beta", (128, D), f32, kind="ExternalInput")
    t_out = nc.dram_tensor("out", (128, nslot, D), bf16,
                           kind="ExternalOutput")

    def chunk_segments(k):
        s0 = 128 * k
        c0 = s0 // L
        if c0 >= NCELL:
            return []
        end0 = L * (c0 + 1)
        if end0 >= s0 + 128 or c0 + 1 >= NCELL:
            return [(0, c0)]
        return [(0, c0), (SUBW, c0 + 1)]

    gstart = [0]
    for g in range(ngroups):
        cend = min((g + 1) * CPG, NCELL)
        gstart.append(min((L * cend) // 128, nch))
    batches = []
    for g in range(ngroups):
        c0, c1 = gstart[g], gstart[g + 1]
        for b in range(c0, c1, RELUB):
            batches.append((b, min(RELUB, c1 - b), g))
    nb = len(batches)
    relu_eng = _relu_pattern(nb, WA, WD)
    first_piece = {}
    last_piece = {}
    for k in range(nch):
        for (mc, cell) in chunk_segments(k):
            if cell not in first_piece:
                first_piece[cell] = (k, mc)
            last_piece[cell] = (k, mc)

    def cell_pos(cell):
        ci = cell % CPG
        wi = ci // 8
        return (64 * (wi // 2), (wi % 2) * 128 + (ci % 8) * SUBW)

    with tile.TileContext(nc) as tc:
        with tc.tile_pool(name="consts", bufs=1) as consts, \
             tc.tile_pool(name="meta", bufs=1) as meta, \
             tc.tile_pool(name="stream", bufs=SBUFS) as stream, \
             tc.tile_pool(name="gpool", bufs=8) as gpool, \
             tc.tile_pool(name="mpool", bufs=2) as mpool, \
             tc.tile_pool(name="ph3", bufs=2) as ph3, \
             tc.tile_pool(name="ps_h", bufs=HBUFS, space="PSUM") as ps_h, \
             tc.tile_pool(name="ps_agg", bufs=ABUFS, space="PSUM") as ps_agg, \
             tc.tile_pool(name="ps_mm", bufs=int(os.environ.get("GNN_MMBUFS", "1")), space="PSUM") as ps_mm, \
             tc.tile_pool(name="ps_nm", bufs=2, space="PSUM") as ps_nm:

            qload = {"p": nc.gpsimd, "s": nc.sync, "a": nc.scalar}[QLOAD]

            def load(pool, t_dram, shape, dt, tag):
                t = pool.tile(shape, dt, tag=tag, name=tag)
                qload.dma_start(out=t[:], in_=t_dram[:])
                return t

            W1B = consts.tile([128, 128], bf16, tag="W1B", name="W1B")
            nc.sync.dma_start(out=W1B[:], in_=t_W1B[:])
            WgpB = load(consts, t_WgpB, [128, 128], bf16, "WgpB")
            WgbB = load(consts, t_WgbB, [128, 128], bf16, "WgbB")
            Wu1pB = load(consts, t_Wu1pB, [128, 128], bf16, "Wu1pB")
            Wu1bB = load(consts, t_Wu1bB, [128, 128], bf16, "Wu1bB")
            Wu2B = load(consts, t_Wu2B, [128, 128], bf16, "Wu2B")
            bu1 = load(consts, t_bu1, [2 * D, 1], f32, "bu1")
            if flags["has_b1"]:
                b1T = load(consts, t_b1T, [128, RELUB * D], f32, "b1T")
                wrow = load(meta, t_wrow, [1, S], bf16, "wrow")
            if flags["has_b2"]:
                b2gRow = load(consts, t_b2gRow, [1, 128], f32, "b2gRow")
                b2uRow = load(consts, t_b2uRow, [1, 128], f32, "b2uRow")
                sumwNM = load(meta, t_sumwNM, [128, nslot], f32, "sumwNM")
            if flags["has_bg"]:
                bgT = load(consts, t_bgT, [128, 128], f32, "bgT")
            if flags["has_bu2"]:
                bu2T = load(consts, t_bu2T, [128, 128], f32, "bu2T")
            if flags["has_gb"]:
                gamma = load(consts, t_gamma, [128, D], f32, "gamma")
                beta = load(consts, t_beta, [128, D], f32, "beta")
            nfT = load(meta, t_nfT, [2 * D, ngroups * 256], bf16, "nfT")
            nfNM = load(meta, t_nfNM, [128, nslot * D], bf16, "nfNM")
            aggHx = meta.tile([128, ngroups * 256], bf16, tag="aggHx")
            X = meta.tile([128, nslot, D], f32, tag="X")
            Xo = meta.tile([128, nslot, D], bf16, tag="Xo")
            mv = meta.tile([128, nslot, 2], f32, tag="mv")
            eps_t = consts.tile([128, 1], f32, tag="eps")
            nc.vector.memset(eps_t[:], LN_EPS)

            nstream = math.ceil(nch / DMACH)
            stream_tiles = {}
            qmap = {"s": nc.sync, "a": nc.scalar, "p": nc.gpsimd}

            def get_stream(si):
                if si in stream_tiles:
                    return stream_tiles[si]
                c0 = si * DMACH
                cn = min(DMACH, nch - c0)
                ncols = (cn + 1) // 2 * 128
                t = stream.tile([128, DMACH // 2 * 128], bf16, tag="nfg",
                                name="nfg")
                q = qmap[QPAT[si % len(QPAT)]]
                q.dma_start(
                    out=t[:, :ncols],
                    in_=t_nfgP[:, c0 // 2 * 128: c0 // 2 * 128 + ncols])
                stream_tiles[si] = t
                return t

            m_tiles = {}
            mg_n = math.ceil(ngroups / MGRPS)
            m_maxw = max(gstart[min(i * MGRPS + MGRPS, ngroups)]
                         - gstart[i * MGRPS] for i in range(mg_n))

            def get_mtile(mi):
                if mi in m_tiles:
                    return m_tiles[mi]
                ck0 = gstart[mi * MGRPS]
                ck1 = gstart[min((mi + 1) * MGRPS, ngroups)]
                t = mpool.tile([128, m_maxw * 2 * SUBW], fp8, tag="M32",
                               name="M32")
                (nc.sync if mi == 0 else qload).dma_start(
                    out=t[:, : (ck1 - ck0) * 2 * SUBW],
                    in_=t_M32[:, ck0 * 2 * SUBW: ck1 * 2 * SUBW])
                m_tiles[mi] = t
                return t

            get_stream(0)
            get_mtile(0)
            if nstream > 1:
                get_stream(1)

            g4_tiles = {}
            agg_tiles = {}

            def emit_front(s):
                c0, cn, g = batches[s]
                if c0 == gstart[g]:
                    agg_tiles[g] = ps_agg.tile([128, 256], f32, tag="aggH",
                                               name="aggH")
                    mi = g // MGRPS
                    if mi + 1 < mg_n and g % MGRPS == 0:
                        get_mtile(mi + 1)
                h_ps = ps_h.tile([128, RELUB * D], f32, tag="h", name="h")
                for c in range(0, cn, 2):
                    ci = c0 + c
                    si = ci // DMACH
                    st = get_stream(si)
                    if ci % DMACH == 0 and si + 2 < nstream:
                        get_stream(si + 2)
                    off = (ci // 2 - si * (DMACH // 2)) * 128
                    nc.tensor.matmul(
                        out=h_ps[:, c * D:(c + 2) * D],
                        lhsT=st[:, off:off + 128],
                        rhs=W1B[:], start=True, stop=True,
                        skip_group_check=True)
                G4 = gpool.tile([128, RELUB, D], bf16, tag="G4", name="G4")
                g4ap = G4[:, :cn, :].rearrange("p a b -> p (a b)")
                if relu_eng[s] == "a":
                    nc.scalar.activation(
                        out=g4ap, in_=h_ps[:, : cn * D],
                        func=mybir.ActivationFunctionType.Relu)
                else:
                    nc.vector.tensor_scalar(
                        out=g4ap, in0=h_ps[:, : cn * D],
                        scalar1=0.0, scalar2=None,
                        op0=mybir.AluOpType.max)
                g4_tiles[s] = G4

            def emit_back(s):
                c0, cn, g = batches[s]
                G4 = g4_tiles.pop(s)
                aggH = agg_tiles[g]
                mi = g // MGRPS
                Mt = m_tiles[mi]
                mbase = gstart[mi * MGRPS]
                for c in range(cn):
                    ci = c0 + c
                    for (mc, cell) in chunk_segments(ci):
                        pb, col = cell_pos(cell)
                        nc.tensor.matmul(
                            out=aggH[pb:pb + D, col:col + SUBW],
                            lhsT=G4[:, c, :],
                            rhs=Mt[:, (ci - mbase) * 2 * SUBW + mc:
                                   (ci - mbase) * 2 * SUBW + mc + SUBW],
                            start=(first_piece[cell] == (ci, mc)),
                            stop=(last_piece[cell] == (ci, mc)),
                            skip_group_check=True)
                if c0 + cn == gstart[g + 1]:
                    emit_phase3(g)

            sqv = meta.tile([128, nslot], f32, tag="sqv")
            rstd = meta.tile([128, nslot], f32, tag="rstd")
            gwl = WPC - (ngroups - 1) * GRP
            real_q = list(range((ngroups - 1) * GRP)) + \
                [(ngroups - 1) * GRP + q for q in
                 ([0, 2, 1, 3][:gwl] if gwl < GRP else range(GRP))]
            padq = [q for q in range(nslot) if q not in set(real_q)]
            for q in padq:
                nc.vector.memset(mv[:, q, :], 1.0)
                nc.gpsimd.memset(X[:, q, :], 0.0)
            p3state = {}

            def stage_a(g):
                gw = min(GRP, WPC - g * GRP)
                aggH = agg_tiles.pop(g)
                gx = g * 256
                if gw <= 2:
                    # bottom partition half never written: zero it so the
                    # block matmuls read finite values
                    nc.vector.memset(aggH[D:, :], 0.0)
                nc.scalar.activation(
                    out=aggHx[:, gx:gx + 256], in_=aggH[:],
                    func=mybir.ActivationFunctionType.Copy)

                u1pre = ps_mm.tile([128, 256], f32, tag="mm", name="u1pre")
                nm2 = ps_nm.tile([128, 2, 256], f32, tag="nm", name="nm2")
                gpre = nm2[:, 0, :]
                for b in range(2):
                    wc = b * 128
                    agg_sl = aggHx[:, gx + wc:gx + wc + 128]
                    nf_sl = nfT[:, gx + wc:gx + wc + 128]
                    nc.tensor.matmul(
                        out=gpre[:, wc:wc + 128],
                        lhsT=agg_sl, rhs=WgpB[:],
                        start=True, stop=False, skip_group_check=True)
                    nc.tensor.matmul(
                        out=gpre[:, wc:wc + 128],
                        lhsT=nf_sl, rhs=WgbB[:],
                        start=False, stop=True,
                        skip_group_check=True)
                    nc.tensor.matmul(
                        out=u1pre[:, wc:wc + 128],
                        lhsT=Wu1pB[:], rhs=agg_sl,
                        start=True, stop=False, skip_group_check=True)
                    nc.tensor.matmul(
                        out=u1pre[:, wc:wc + 128],
                        lhsT=Wu1bB[:], rhs=nf_sl,
                        start=False, stop=True, skip_group_check=True)
                p3state[g] = (u1pre, nm2)

            def stage_b(g):
                u1pre, nm2 = p3state[g]
                gpre = nm2[:, 0, :]
                u2pre = nm2[:, 1, :]
                u1T = ph3.tile([128, 256], bf16, tag="u1T")
                nc.scalar.activation(
                    out=u1T[:], in_=u1pre[:],
                    func=mybir.ActivationFunctionType.Relu, bias=bu1[:])
                for b in range(2):
                    wc = b * 128
                    nc.tensor.matmul(
                        out=u2pre[:, wc:wc + 128],
                        lhsT=u1T[:, wc:wc + 128],
                        rhs=Wu2B[:], start=True, stop=True,
                        skip_group_check=True)
                if flags["has_bg"]:
                    nc.vector.tensor_tensor(
                        out=gpre[:, 0:128], in0=gpre[:, 0:128],
                        in1=bgT[:], op=mybir.AluOpType.add)
                    nc.vector.tensor_tensor(
                        out=gpre[:, 128:256], in0=gpre[:, 128:256],
                        in1=bgT[:], op=mybir.AluOpType.add)
                if flags["has_bu2"]:
                    nc.vector.tensor_tensor(
                        out=u2pre[:, 0:128], in0=u2pre[:, 0:128],
                        in1=bu2T[:], op=mybir.AluOpType.add)
                    nc.vector.tensor_tensor(
                        out=u2pre[:, 128:256], in0=u2pre[:, 128:256],
                        in1=bu2T[:], op=mybir.AluOpType.add)
                gateT = ph3.tile([128, 256], bf16, tag="gateT")
                nc.scalar.activation(
                    out=gateT[:], in_=gpre[:],
                    func=mybir.ActivationFunctionType.Sigmoid)
                p3state[g] = (nm2, gateT)

            def stage_c(g):
                gw = min(GRP, WPC - g * GRP)
                nm2, gateT = p3state.pop(g)
                u2pre = nm2[:, 1, :]
                q0 = g * GRP
                nfg_nm = nfNM[:, q0 * D: (q0 + GRP) * D]
                Xg = X[:, q0:q0 + GRP, :].rearrange("p a b -> p (a b)")
                nc.vector.tensor_tensor(
                    out=Xg, in0=u2pre[:], in1=nfg_nm,
                    op=mybir.AluOpType.subtract)
                nc.gpsimd.tensor_tensor(
                    out=Xg, in0=Xg, in1=gateT[:],
                    op=mybir.AluOpType.mult)
                nc.gpsimd.tensor_tensor(
                    out=Xg, in0=Xg, in1=nfg_nm,
                    op=mybir.AluOpType.add)
                stats = ph3.tile([128, GRP, 6], f32, tag="stats")
                qw = [0, 2, 1, 3][:gw] if gw < GRP else range(GRP)
                for q in qw:
                    nc.vector.bn_stats(out=stats[:, q, :],
                                       in_=X[:, q0 + q, :])
                    nc.vector.bn_aggr(out=mv[:, q0 + q, :],
                                      in_=stats[:, q, :])

            def emit_phase3(g):
                stage_a(g)

            SPLITG = int(os.environ.get("GNN_SPLITG", "18"))

            def flush(qlo, qhi):
                var_ap = bass.AP(
                    tensor=mv.tensor, offset=mv[:, qlo:, 1:2].offset,
                    ap=[mv[:].ap[0], [2, qhi - qlo]])
                nc.scalar.activation(
                    out=sqv[:, qlo:qhi], in_=var_ap,
                    func=mybir.ActivationFunctionType.Sqrt, bias=eps_t[:])
                nc.vector.reciprocal(out=rstd[:, qlo:qhi],
                                     in_=sqv[:, qlo:qhi])
                for q in range(qlo, qhi):
                    eng = nc.gpsimd
                    eng.tensor_scalar(
                        out=Xo[:, q, :], in0=X[:, q, :],
                        scalar1=mv[:, q, 0:1], scalar2=rstd[:, q:q + 1],
                        op0=mybir.AluOpType.subtract,
                        op1=mybir.AluOpType.mult)
                if flags["has_gb"]:
                    nn = qhi - qlo
                    gam_b = bass.AP(
                        tensor=gamma.tensor, offset=gamma[:].offset,
                        ap=[gamma[:].ap[0], [0, nn], [1, D]])
                    bet_b = bass.AP(
                        tensor=beta.tensor, offset=beta[:].offset,
                        ap=[beta[:].ap[0], [0, nn], [1, D]])
                    Xsl = Xo[:, qlo:qhi, :].rearrange("p a b -> p (a b)")
                    nc.vector.tensor_tensor(
                        out=Xsl, in0=Xsl, in1=gam_b, op=mybir.AluOpType.mult)
                    nc.vector.tensor_tensor(
                        out=Xsl, in0=Xsl, in1=bet_b, op=mybir.AluOpType.add)
                nc.sync.dma_start(out=t_out[:, qlo:qhi, :],
                                  in_=Xo[:, qlo:qhi, :])

            # batch index at which each group's chunks complete
            gend = {}
            for i, (c0, cn, g) in enumerate(batches):
                gend[g] = i
            due_b = {gend[g] + SB: g for g in range(ngroups)}
            due_c = {gend[g] + SC: g for g in range(ngroups)}

            for s in range(nb + LOOK + SC + 1):
                if s < nb:
                    emit_front(s)
                if s >= LOOK and s - LOOK < nb:
                    emit_back(s - LOOK)
                sl = s - LOOK
                if sl in due_b:
                    stage_b(due_b[sl])
                if sl in due_c:
                    stage_c(due_c[sl])
                    if due_c[sl] == SPLITG:
                        flush(0, (SPLITG + 1) * GRP)

            # ---- LN tail: flush remaining slots ----
            flush((SPLITG + 1) * GRP, nslot)
    nc.compile()
    return nc


def kernel(**inputs) -> np.ndarray:
    node_feats = np.asarray(inputs["node_feats"], np.float32)
    neighbor_feats = np.asarray(inputs["neighbor_feats"], np.float32)
    per_core, L, nch = _host_prep(
        node_feats, neighbor_feats,
        inputs["edge_indices"], np.asarray(inputs["edge_weights"], np.float32))
    consts, flags = _const_inputs(
        inputs["W1"], inputs["b1"], inputs["W2"], inputs["b2"],
        inputs["Wg"], inputs["bg"], inputs["Wu1"], inputs["bu1"],
        inputs["Wu2"], inputs["bu2"], inputs["gamma"], inputs["beta"])
    if flags["has_b1"] or flags["has_b2"]:
        raise NotImplementedError("nonzero b1/b2 path not wired")

    key = ("nc5", L, nch, tuple(sorted(flags.items())))
    if key not in _CACHE:
        _CACHE.clear()
        _CACHE[key] = _build_nc(L, nch, flags)
    nc = _CACHE[key]

    in_maps = []
    colmaps = []
    for k in range(NCORES):
        m = dict(consts)
        pc = dict(per_core[k])
        colmaps.append(pc.pop("_colmap"))
        sw = pc.pop("_sumwNM")
        wr = pc.pop("_wrow")
        if flags["has_b2"]:
            pc["sumwNM"] = sw
        m.update(pc)
        in_maps.append(m)
    res = run_bass_kernel_spmd(nc, in_maps, core_ids=list(range(NCORES)))
    out = np.empty((N, D), np.float32)
    for k in range(NCORES):
        y = np.asarray(res.results[k]["out"], dtype=np.float32)
        lo = k * NPC
        hi = min(lo + NPC, N)
        nslot = y.shape[1]
        full = y.transpose(1, 0, 2).reshape(nslot * 128, D)
        out[lo:hi] = full[colmaps[k][: hi - lo]]
    return out


# revision 4
# speedup vs baseline: 1.0032x; 1.0032x over previous
"""Trainium2 Bass kernel for nn_MessagePassingLayer (GNN message passing), v5.

Like v4 (host LPT cell packing ~4% padding, pair-packed [128, S/2] bf16
stream, host-built fp8 one-hot scatter blocks, W2 folded into gate/update
weights, node-major phase 3, software-pipelined PE stream, batched-rstd
LayerNorm tail) but with one crucial hardware fix:

Matmuls whose operands sit at partition base 64 crash the runtime when
interleaved with base-0 matmuls (PE quadrant switching). So every matmul
here contracts over all 128 partitions with BLOCK-DIAGONAL weights:
- stream pairs: h for chunks (2P, 2P+1) = stream_block[128, 128] @
  blockdiag(W1, W1) -> [128 slots, 128] (two 64-col h halves).
- phase 3 pairs windows (w, w+2) on partition halves of the packed
  aggregate/nfT tiles and uses blockdiag(Wgp), blockdiag(Wgb), etc.
Window slots are emitted in memory order [w0, w2, w1, w3] per group; the
host unscrambles via the column map. X/Xo carry 4*ngroups slots (100).
"""
import heapq
import math
import os
import numpy as np
import ml_dtypes

import concourse.bass as bass
import concourse.bacc as bacc
import concourse.tile as tile
from concourse import mybir
from concourse.bass_utils import run_bass_kernel_spmd

BF = ml_dtypes.bfloat16
F8 = ml_dtypes.float8_e4m3
N = 100000
D = 64
E = 1250000
NCORES = 8
NPC = 12544
WPC = 98
SUBW = 16
NCELL = NPC // SUBW
LN_EPS = 1e-3
GRP = 4
CPG = GRP * 128 // SUBW  # 32

_CACHE = {}


def _pack_cells(deg):
    order = np.argsort(-deg, kind="stable")
    heap = [(0, 0, c) for c in range(NCELL)]
    heapq.heapify(heap)
    cell_id = np.empty(NPC, np.int32)
    pos = np.empty(NPC, np.int32)
    for i in order:
        s, cnt, c = heapq.heappop(heap)
        cell_id[i] = c
        pos[i] = cnt
        if cnt + 1 < SUBW:
            heapq.heappush(heap, (s + int(deg[i]), cnt + 1, c))
    return cell_id, pos


def _host_prep(node_feats, neighbor_feats, edge_indices, edge_weights):
    src = np.asarray(edge_indices[0], dtype=np.int64)
    dst = np.asarray(edge_indices[1], dtype=np.int64)
    w = np.asarray(edge_weights[:, 0], dtype=np.float64)

    wdeg = np.bincount(dst, weights=w, minlength=N)
    recdeg = 1.0 / np.maximum(wdeg, 1.0)
    wprime = (w * recdeg[dst]).astype(np.float32)
    sumw = (wdeg * recdeg).astype(np.float32)

    cnt = np.zeros(NCORES * NPC, np.int64)
    cnt[:N] = np.bincount(dst, minlength=N)

    cell_id = np.empty(NCORES * NPC, np.int32)
    pos_in = np.empty(NCORES * NPC, np.int32)
    maxcell = 0
    for k in range(NCORES):
        lo = k * NPC
        cid, p = _pack_cells(cnt[lo:lo + NPC])
        cell_id[lo:lo + NPC] = cid
        pos_in[lo:lo + NPC] = p
        sums = np.bincount(cid, weights=cnt[lo:lo + NPC], minlength=NCELL)
        maxcell = max(maxcell, int(sums.max()))
    L = max(16 * math.ceil(maxcell / 16), 144)
    nch = NCELL * L // 128
    if nch % 2:
        nch += 1
    S = nch * 128
    ngroups = math.ceil(WPC / GRP)
    nslot = ngroups * GRP          # 100 window slots (2 padding)

    core = dst // NPC
    ecell = cell_id[dst]
    epos = pos_in[dst]

    gcell = core.astype(np.int64) * NCELL + ecell
    order = np.argsort(gcell, kind="stable")
    gc_s = gcell[order]
    counts = np.bincount(gc_s, minlength=NCORES * NCELL)
    starts = np.zeros(NCORES * NCELL + 1, np.int64)
    starts[1:] = np.cumsum(counts)
    rank = np.arange(E) - starts[gc_s]
    slot = (gc_s % NCELL) * L + rank
    src_s = src[order]
    pos_s = epos[order]
    wp_s = wprime[order]
    core_s = core[order]

    nfeats = np.asarray(neighbor_feats, dtype=np.float32)
    node_f = np.asarray(node_feats, dtype=np.float32)

    # cell c (global, per core) geometry:
    #   group g = c // CPG, ci = c % CPG, window wi = ci // 8 (0..3),
    #   j = ci % 8. Memory: partition half = 64*(ci//16) [= wi//2],
    #   node-col within group block = (wi%2)*128 + j*16 + pos.
    # window slot order per group: q(wi) = (wi%2)*2 + wi//2  ([0,2,1,3]).
    cells = np.arange(NCELL)
    cg = cells // CPG
    ci = cells % CPG
    wi = ci // 8
    jj = ci % 8
    qslot = cg * GRP + (wi % 2) * 2 + (wi // 2)     # memory window slot
    half = (wi // 2)                                 # 0 or 1 (partition)
    # node-major output position: Xo[p, qslot, :] rows p = node col in win
    # node col within window = j*16 + pos
    # colmap entry for node: qslot*128 + j*16 + pos
    cell_q = qslot
    cell_j = jj
    colmap_cell = cell_q * 128 + cell_j * SUBW       # + pos

    # per-cell aggregation PSUM position within group tile [128, 256]:
    #   partitions 64*half + feat, cols (wi%2)*128 + j*16
    agg_pb = 64 * half
    agg_col = (wi % 2) * 128 + jj * SUBW

    per_core = []
    for k in range(NCORES):
        m = core_s == k
        sl = slot[m]
        nfg = np.zeros((S, D), np.float32)
        nfg[sl] = nfeats[src_s[m]] * wp_s[m][:, None]
        nfgP = np.ascontiguousarray(
            nfg.reshape(nch // 2, 2, 128, D).transpose(1, 3, 0, 2)
            .reshape(128, S // 2))

        chunk_of = sl // 128
        first_cell = (chunk_of * 128) // L
        ecl = sl // L
        mcol = (pos_s[m] + SUBW * (ecl - first_cell)).astype(np.int64)
        M = np.zeros((nch, 128, 2 * SUBW), np.float32)
        M[chunk_of, sl % 128, mcol] = 1.0
        M32 = np.ascontiguousarray(M.transpose(1, 0, 2)).reshape(
            128, nch * 2 * SUBW)

        wr = np.zeros((1, S), np.float32)
        wr[0, sl] = wp_s[m]

        lo = k * NPC
        hi = min(lo + NPC, N)
        nfp = np.zeros((NPC, D), np.float32)
        nfp[: hi - lo] = node_f[lo:hi]
        nl = np.arange(hi - lo)
        cell_n = cell_id[lo:lo + NPC][: hi - lo]
        pos_n = pos_in[lo:lo + NPC][: hi - lo]
        # packed feature-major nfT [128, ngroups*256]:
        #   partition 64*half + feat, col g*256 + (wi%2)*128 + j*16 + pos
        g_n = cell_n // CPG
        ci_n = cell_n % CPG
        wi_n = ci_n // 8
        j_n = ci_n % 8
        half_n = wi_n // 2
        colT = g_n * 256 + (wi_n % 2) * 128 + j_n * SUBW + pos_n
        ngroups_ = ngroups
        nfT = np.zeros((2 * D, ngroups_ * 256), np.float32)
        nfT[(half_n[None, :] * D + np.arange(D)[:, None]),
            colT[None, :]] = nfp[: hi - lo].T
        # node-major nfNM [128, nslot*64]: row = node col in window,
        # col = qslot*64 + feat
        q_n = g_n * GRP + (wi_n % 2) * 2 + half_n
        row_n = j_n * SUBW + pos_n
        nfNM = np.zeros((128, nslot * D), np.float32)
        nfNM[row_n[:, None],
             (q_n * D)[:, None] + np.arange(D)[None, :]] = nfp[: hi - lo]
        swv = np.zeros((128, nslot), np.float32)
        swv[row_n, q_n] = sumw[lo:hi][: hi - lo]
        cm = (q_n * 128 + row_n)
        per_core.append({
            "nfgP": nfgP.astype(BF),
            "M32": M32.astype(F8),
            "nfT": nfT.astype(BF),
            "nfNM": nfNM.astype(BF),
            "_sumwNM": swv.astype(np.float32),   # [128, nslot] if b2 != 0
            "_wrow": wr.astype(BF),
            "_colmap": cm,
        })
    return per_core, L, nch


def _const_inputs(W1, b1, W2, b2, Wg, bg, Wu1, bu1, Wu2, bu2, gamma, beta):
    flags = {
        "has_b1": bool(np.any(np.asarray(b1) != 0)),
        "has_b2": bool(np.any(np.asarray(b2) != 0)),
        "has_bg": bool(np.any(np.asarray(bg) != 0)),
        "has_bu2": bool(np.any(np.asarray(bu2) != 0)),
        "has_gb": bool(np.any(np.asarray(gamma) != 1)
                       or np.any(np.asarray(beta) != 0)),
    }
    W2f = np.asarray(W2, np.float64)
    Wgf = np.asarray(Wg, np.float64)
    Wu1f = np.asarray(Wu1, np.float64)

    def blk(a):
        a = np.asarray(a, np.float32)
        z = np.zeros_like(a)
        return np.block([[a, z], [z, a]]).astype(BF)   # [128, 128]

    c = {
        "W1B": blk(W1),
        "WgpB": blk((W2f @ Wgf[:D]).astype(np.float32)),
        "WgbB": blk(Wgf[D:].astype(np.float32)),
        "Wu1pB": blk((W2f @ Wu1f[:D]).astype(np.float32)),
        "Wu1bB": blk(Wu1f[D:].astype(np.float32)),
        "Wu2B": blk(Wu2),
        "bu1": np.tile(np.asarray(bu1, np.float32), 2)[:, None],
    }
    # general-bias fallbacks (zero in this model): materialized lane tiles
    if flags["has_bg"]:
        bgp = np.tile(np.asarray(bg, np.float32), 2)  # gpre block layout
        c["bgT"] = np.tile(bgp[None, :], (128, 1))    # [128, 128]
    if flags["has_bu2"]:
        b2p = np.tile(np.asarray(bu2, np.float32), 2)
        c["bu2T"] = np.tile(b2p[None, :], (128, 1))
    if flags["has_b2"]:
        c["b2gRow"] = np.tile((np.asarray(b2, np.float64) @ Wgf[:D])
                              .astype(np.float32), 2)[None, :]  # [1,128]
        c["b2uRow"] = np.tile((np.asarray(b2, np.float64) @ Wu1f[:D])
                              .astype(np.float32), 2)[None, :]
    if flags["has_b1"]:
        c["b1T"] = np.tile(np.asarray(b1, np.float32)[None, :], (128, 8))
    if flags["has_gb"]:
        c["gamma"] = np.tile(np.asarray(gamma, np.float32)[None, :], (128, 1))
        c["beta"] = np.tile(np.asarray(beta, np.float32)[None, :], (128, 1))
    return c, flags


def _relu_pattern(n, wa, wd):
    out = []
    acc_a = acc_d = 0.0
    for _ in range(n):
        acc_a += wa
        acc_d += wd
        if acc_a >= acc_d:
            acc_a -= wa + wd
            out.append("a")
        else:
            acc_d -= wa + wd
            out.append("d")
    return out


def _build_nc(L, nch, flags):
    RELUB = int(os.environ.get("GNN_RELUB", "8"))
    DMACH = int(os.environ.get("GNN_DMACH", "32"))
    SBUFS = int(os.environ.get("GNN_SBUFS", "5"))
    QPAT = os.environ.get("GNN_QPAT", "s")
    WA = int(os.environ.get("GNN_WA", "100"))
    WD = int(os.environ.get("GNN_WD", "100"))
    HBUFS = int(os.environ.get("GNN_HBUFS", "3"))
    ABUFS = int(os.environ.get("GNN_ABUFS", "1"))
    LOOK = int(os.environ.get("GNN_LOOK", "4"))
    SB = int(os.environ.get("GNN_SB", "3"))
    SC = int(os.environ.get("GNN_SC", "7"))
    MGRPS = int(os.environ.get("GNN_MGRPS", "4"))
    QLOAD = os.environ.get("GNN_QLOAD", "p")
    bf16, f32 = mybir.dt.bfloat16, mybir.dt.float32
    fp8 = mybir.dt.float8e4
    S = nch * 128
    assert nch % 2 == 0 and DMACH % 2 == 0 and RELUB % 2 == 0
    nc = bacc.Bacc("TRN2", target_bir_lowering=False, debug=False)

    ngroups = math.ceil(WPC / GRP)
    nslot = ngroups * GRP

    t_nfgP = nc.dram_tensor("nfgP", (128, S // 2), bf16, kind="ExternalInput")
    t_M32 = nc.dram_tensor("M32", (128, nch * 2 * SUBW), fp8,
                           kind="ExternalInput")
    t_nfT = nc.dram_tensor("nfT", (2 * D, ngroups * 256), bf16,
                           kind="ExternalInput")
    t_nfNM = nc.dram_tensor("nfNM", (128, nslot * D), bf16,
                            kind="ExternalInput")
    t_W1B = nc.dram_tensor("W1B", (128, 128), bf16, kind="ExternalInput")
    t_WgpB = nc.dram_tensor("WgpB", (128, 128), bf16, kind="ExternalInput")
    t_WgbB = nc.dram_tensor("WgbB", (128, 128), bf16, kind="ExternalInput")
    t_Wu1pB = nc.dram_tensor("Wu1pB", (128, 128), bf16, kind="ExternalInput")
    t_Wu1bB = nc.dram_tensor("Wu1bB", (128, 128), bf16, kind="ExternalInput")
    t_Wu2B = nc.dram_tensor("Wu2B", (128, 128), bf16, kind="ExternalInput")
    t_bu1 = nc.dram_tensor("bu1", (2 * D, 1), f32, kind="ExternalInput")
    if flags["has_b1"]:
        t_b1T = nc.dram_tensor("b1T", (128, RELUB * D), f32,
                               kind="ExternalInput")
        t_wrow = nc.dram_tensor("wrow", (1, S), bf16, kind="ExternalInput")
    if flags["has_b2"]:
        t_b2gRow = nc.dram_tensor("b2gRow", (1, 128), f32,
                                  kind="ExternalInput")
        t_b2uRow = nc.dram_tensor("b2uRow", (1, 128), f32,
                                  kind="ExternalInput")
        t_sumwNM = nc.dram_tensor("sumwNM", (128, nslot), f32,
                                  kind="ExternalInput")
    if flags["has_bg"]:
        t_bgT = nc.dram_tensor("bgT", (128, 128), f32, kind="ExternalInput")
    if flags["has_bu2"]:
        t_bu2T = nc.dram_tensor("bu2T", (128, 128), f32, kind="ExternalInput")
    if flags["has_gb"]:
        t_gamma = nc.dram_tensor("gamma", (128, D), f32, kind="ExternalInput")
        t_beta = nc.dram_tensor("beta", (128, D), f32, kind="ExternalInput")
    t_out = nc.dram_tensor("out", (128, nslot, D), bf16,
                           kind="ExternalOutput")

    def chunk_segments(k):
        s0 = 128 * k
        c0 = s0 // L
        if c0 >= NCELL:
            return []
        end0 = L * (c0 + 1)
        if end0 >= s0 + 128 or c0 + 1 >= NCELL:
            return [(0, c0)]
        return [(0, c0), (SUBW, c0 + 1)]

    gstart = [0]
    for g in range(ngroups):
        cend = min((g + 1) * CPG, NCELL)
        gstart.append(min((L * cend) // 128, nch))
    batches = []
    for g in range(ngroups):
        c0, c1 = gstart[g], gstart[g + 1]
        for b in range(c0, c1, RELUB):
            batches.append((b, min(RELUB, c1 - b), g))
    nb = len(batches)
    relu_eng = _relu_pattern(nb, WA, WD)
    first_piece = {}
    last_piece = {}
    for k in range(nch):
        for (mc, cell) in chunk_segments(k):
            if cell not in first_piece:
                first_piece[cell] = (k, mc)
            last_piece[cell] = (k, mc)

    def cell_pos(cell):
        ci = cell % CPG
        wi = ci // 8
        return (64 * (wi // 2), (wi % 2) * 128 + (ci % 8) * SUBW)

    with tile.TileContext(nc) as tc:
        with tc.tile_pool(name="consts", bufs=1) as consts, \
             tc.tile_pool(name="meta", bufs=1) as meta, \
             tc.tile_pool(name="stream", bufs=SBUFS) as stream, \
             tc.tile_pool(name="gpool", bufs=8) as gpool, \
             tc.tile_pool(name="mpool", bufs=2) as mpool, \
             tc.tile_pool(name="ph3", bufs=2) as ph3, \
             tc.tile_pool(name="ps_h", bufs=HBUFS, space="PSUM") as ps_h, \
             tc.tile_pool(name="ps_agg", bufs=ABUFS, space="PSUM") as ps_agg, \
             tc.tile_pool(name="ps_mm", bufs=int(os.environ.get("GNN_MMBUFS", "1")), space="PSUM") as ps_mm, \
             tc.tile_pool(name="ps_nm", bufs=2, space="PSUM") as ps_nm:

            qload = {"p": nc.gpsimd, "s": nc.sync, "a": nc.scalar}[QLOAD]

            def load(pool, t_dram, shape, dt, tag):
                t = pool.tile(shape, dt, tag=tag, name=tag)
                qload.dma_start(out=t[:], in_=t_dram[:])
                return t

            W1B = consts.tile([128, 128], bf16, tag="W1B", name="W1B")
            nc.sync.dma_start(out=W1B[:], in_=t_W1B[:])
            WgpB = load(consts, t_WgpB, [128, 128], bf16, "WgpB")
            WgbB = load(consts, t_WgbB, [128, 128], bf16, "WgbB")
            Wu1pB = load(consts, t_Wu1pB, [128, 128], bf16, "Wu1pB")
            Wu1bB = load(consts, t_Wu1bB, [128, 128], bf16, "Wu1bB")
            Wu2B = load(consts, t_Wu2B, [128, 128], bf16, "Wu2B")
            bu1 = load(consts, t_bu1, [2 * D, 1], f32, "bu1")
            if flags["has_b1"]:
                b1T = load(consts, t_b1T, [128, RELUB * D], f32, "b1T")
                wrow = load(meta, t_wrow, [1, S], bf16, "wrow")
            if flags["has_b2"]:
                b2gRow = load(consts, t_b2gRow, [1, 128], f32, "b2gRow")
                b2uRow = load(consts, t_b2uRow, [1, 128], f32, "b2uRow")
                sumwNM = load(meta, t_sumwNM, [128, nslot], f32, "sumwNM")
            if flags["has_bg"]:
                bgT = load(consts, t_bgT, [128, 128], f32, "bgT")
            if flags["has_bu2"]:
                bu2T = load(consts, t_bu2T, [128, 128], f32, "bu2T")
            if flags["has_gb"]:
                gamma = load(consts, t_gamma, [128, D], f32, "gamma")
                beta = load(consts, t_beta, [128, D], f32, "beta")
            nfT = load(meta, t_nfT, [2 * D, ngroups * 256], bf16, "nfT")
            nfNM = load(meta, t_nfNM, [128, nslot * D], bf16, "nfNM")
            aggHx = meta.tile([128, ngroups * 256], bf16, tag="aggHx")
            X = meta.tile([128, nslot, D], f32, tag="X")
            Xo = meta.tile([128, nslot, D], bf16, tag="Xo")
            mv = meta.tile([128, nslot, 2], f32, tag="mv")
            eps_t = consts.tile([128, 1], f32, tag="eps")
            nc.vector.memset(eps_t[:], LN_EPS)

            nstream = math.ceil(nch / DMACH)
            stream_tiles = {}
            qmap = {"s": nc.sync, "a": nc.scalar, "p": nc.gpsimd}

            def get_stream(si):
                if si in stream_tiles:
                    return stream_tiles[si]
                c0 = si * DMACH
                cn = min(DMACH, nch - c0)
                ncols = (cn + 1) // 2 * 128
                t = stream.tile([128, DMACH // 2 * 128], bf16, tag="nfg",
                                name="nfg")
                q = qmap[QPAT[si % len(QPAT)]]
                base = c0 // 2 * 128
                if si == 0 and ncols > 512:
                    # split the first load so compute can start sooner
                    q.dma_start(out=t[:, :512],
                                in_=t_nfgP[:, base: base + 512])
                    q.dma_start(out=t[:, 512:ncols],
                                in_=t_nfgP[:, base + 512: base + ncols])
                else:
                    q.dma_start(out=t[:, :ncols],
                                in_=t_nfgP[:, base: base + ncols])
                stream_tiles[si] = t
                return t

            m_tiles = {}
            mg_n = math.ceil(ngroups / MGRPS)
            m_maxw = max(gstart[min(i * MGRPS + MGRPS, ngroups)]
                         - gstart[i * MGRPS] for i in range(mg_n))

            def get_mtile(mi):
                if mi in m_tiles:
                    return m_tiles[mi]
                ck0 = gstart[mi * MGRPS]
                ck1 = gstart[min((mi + 1) * MGRPS, ngroups)]
                t = mpool.tile([128, m_maxw * 2 * SUBW], fp8, tag="M32",
                               name="M32")
                (nc.sync if mi == 0 else qload).dma_start(
                    out=t[:, : (ck1 - ck0) * 2 * SUBW],
                    in_=t_M32[:, ck0 * 2 * SUBW: ck1 * 2 * SUBW])
                m_tiles[mi] = t
                return t

            get_stream(0)
            get_mtile(0)
            if nstream > 1:
                get_stream(1)

            g4_tiles = {}
            agg_tiles = {}

            def emit_front(s):
                c0, cn, g = batches[s]
                if c0 == gstart[g]:
                    agg_tiles[g] = ps_agg.tile([128, 256], f32, tag="aggH",
                                               name="aggH")
                    mi = g // MGRPS
                    if mi + 1 < mg_n and g % MGRPS == 0:
                        get_mtile(mi + 1)
                h_ps = ps_h.tile([128, RELUB * D], f32, tag="h", name="h")
                for c in range(0, cn, 2):
                    ci = c0 + c
                    si = ci // DMACH
                    st = get_stream(si)
                    if ci % DMACH == 0 and si + 2 < nstream:
                        get_stream(si + 2)
                    off = (ci // 2 - si * (DMACH // 2)) * 128
                    nc.tensor.matmul(
                        out=h_ps[:, c * D:(c + 2) * D],
                        lhsT=st[:, off:off + 128],
                        rhs=W1B[:], start=True, stop=True,
                        skip_group_check=True)
                G4 = gpool.tile([128, RELUB, D], bf16, tag="G4", name="G4")
                g4ap = G4[:, :cn, :].rearrange("p a b -> p (a b)")
                if relu_eng[s] == "a":
                    nc.scalar.activation(
                        out=g4ap, in_=h_ps[:, : cn * D],
                        func=mybir.ActivationFunctionType.Relu)
                else:
                    nc.vector.tensor_scalar(
                        out=g4ap, in0=h_ps[:, : cn * D],
                        scalar1=0.0, scalar2=None,
                        op0=mybir.AluOpType.max)
                g4_tiles[s] = G4

            def emit_back(s):
                c0, cn, g = batches[s]
                G4 = g4_tiles.pop(s)
                aggH = agg_tiles[g]
                mi = g // MGRPS
                Mt = m_tiles[mi]
                mbase = gstart[mi * MGRPS]
                for c in range(cn):
                    ci = c0 + c
                    for (mc, cell) in chunk_segments(ci):
                        pb, col = cell_pos(cell)
                        nc.tensor.matmul(
                            out=aggH[pb:pb + D, col:col + SUBW],
                            lhsT=G4[:, c, :],
                            rhs=Mt[:, (ci - mbase) * 2 * SUBW + mc:
                                   (ci - mbase) * 2 * SUBW + mc + SUBW],
                            start=(first_piece[cell] == (ci, mc)),
                            stop=(last_piece[cell] == (ci, mc)),
                            skip_group_check=True)
                if c0 + cn == gstart[g + 1]:
                    emit_phase3(g)

            sqv = meta.tile([128, nslot], f32, tag="sqv")
            rstd = meta.tile([128, nslot], f32, tag="rstd")
            gwl = WPC - (ngroups - 1) * GRP
            real_q = list(range((ngroups - 1) * GRP)) + \
                [(ngroups - 1) * GRP + q for q in
                 ([0, 2, 1, 3][:gwl] if gwl < GRP else range(GRP))]
            padq = [q for q in range(nslot) if q not in set(real_q)]
            for q in padq:
                nc.vector.memset(mv[:, q, :], 1.0)
                nc.gpsimd.memset(X[:, q, :], 0.0)
            p3state = {}

            def stage_a(g):
                gw = min(GRP, WPC - g * GRP)
                aggH = agg_tiles.pop(g)
                gx = g * 256
                if gw <= 2:
                    # bottom partition half never written: zero it so the
                    # block matmuls read finite values
                    nc.vector.memset(aggH[D:, :], 0.0)
                if os.environ.get("GNN_AGGALT", "0") == "1" and g % 2 == 1:
                    nc.vector.tensor_copy(
                        out=aggHx[:, gx:gx + 256], in_=aggH[:])
                else:
                    nc.scalar.activation(
                        out=aggHx[:, gx:gx + 256], in_=aggH[:],
                        func=mybir.ActivationFunctionType.Copy)

                u1pre = ps_mm.tile([128, 256], f32, tag="mm", name="u1pre")
                nm2 = ps_nm.tile([128, 2, 256], f32, tag="nm", name="nm2")
                gpre = nm2[:, 0, :]
                for b in range(2):
                    wc = b * 128
                    agg_sl = aggHx[:, gx + wc:gx + wc + 128]
                    nf_sl = nfT[:, gx + wc:gx + wc + 128]
                    nc.tensor.matmul(
                        out=gpre[:, wc:wc + 128],
                        lhsT=agg_sl, rhs=WgpB[:],
                        start=True, stop=False, skip_group_check=True)
                    nc.tensor.matmul(
                        out=gpre[:, wc:wc + 128],
                        lhsT=nf_sl, rhs=WgbB[:],
                        start=False, stop=True,
                        skip_group_check=True)
                    nc.tensor.matmul(
                        out=u1pre[:, wc:wc + 128],
                        lhsT=Wu1pB[:], rhs=agg_sl,
                        start=True, stop=False, skip_group_check=True)
                    nc.tensor.matmul(
                        out=u1pre[:, wc:wc + 128],
                        lhsT=Wu1bB[:], rhs=nf_sl,
                        start=False, stop=True, skip_group_check=True)
                p3state[g] = (u1pre, nm2)

            def stage_b(g):
                u1pre, nm2 = p3state[g]
                gpre = nm2[:, 0, :]
                u2pre = nm2[:, 1, :]
                u1T = ph3.tile([128, 256], bf16, tag="u1T")
                nc.scalar.activation(
                    out=u1T[:], in_=u1pre[:],
                    func=mybir.ActivationFunctionType.Relu, bias=bu1[:])
                for b in range(2):
                    wc = b * 128
                    nc.tensor.matmul(
                        out=u2pre[:, wc:wc + 128],
                        lhsT=u1T[:, wc:wc + 128],
                        rhs=Wu2B[:], start=True, stop=True,
                        skip_group_check=True)
                if flags["has_bg"]:
                    nc.vector.tensor_tensor(
                        out=gpre[:, 0:128], in0=gpre[:, 0:128],
                        in1=bgT[:], op=mybir.AluOpType.add)
                    nc.vector.tensor_tensor(
                        out=gpre[:, 128:256], in0=gpre[:, 128:256],
                        in1=bgT[:], op=mybir.AluOpType.add)
                if flags["has_bu2"]:
                    nc.vector.tensor_tensor(
                        out=u2pre[:, 0:128], in0=u2pre[:, 0:128],
                        in1=bu2T[:], op=mybir.AluOpType.add)
                    nc.vector.tensor_tensor(
                        out=u2pre[:, 128:256], in0=u2pre[:, 128:256],
                        in1=bu2T[:], op=mybir.AluOpType.add)
                gateT = ph3.tile([128, 256], bf16, tag="gateT")
                nc.scalar.activation(
                    out=gateT[:], in_=gpre[:],
                    func=mybir.ActivationFunctionType.Sigmoid)
                p3state[g] = (nm2, gateT)

            def stage_c(g):
                gw = min(GRP, WPC - g * GRP)
                nm2, gateT = p3state.pop(g)
                u2pre = nm2[:, 1, :]
                q0 = g * GRP
                nfg_nm = nfNM[:, q0 * D: (q0 + GRP) * D]
                Xg = X[:, q0:q0 + GRP, :].rearrange("p a b -> p (a b)")
                nc.vector.tensor_tensor(
                    out=Xg, in0=u2pre[:], in1=nfg_nm,
                    op=mybir.AluOpType.subtract)
                nc.gpsimd.tensor_tensor(
                    out=Xg, in0=Xg, in1=gateT[:],
                    op=mybir.AluOpType.mult)
                nc.gpsimd.tensor_tensor(
                    out=Xg, in0=Xg, in1=nfg_nm,
                    op=mybir.AluOpType.add)
                stats = ph3.tile([128, GRP, 6], f32, tag="stats")
                qw = [0, 2, 1, 3][:gw] if gw < GRP else range(GRP)
                for q in qw:
                    nc.vector.bn_stats(out=stats[:, q, :],
                                       in_=X[:, q0 + q, :])
                    nc.vector.bn_aggr(out=mv[:, q0 + q, :],
                                      in_=stats[:, q, :])

            def emit_phase3(g):
                stage_a(g)

            SPLITG = int(os.environ.get("GNN_SPLITG", "18"))

            def flush(qlo, qhi):
                var_ap = bass.AP(
                    tensor=mv.tensor, offset=mv[:, qlo:, 1:2].offset,
                    ap=[mv[:].ap[0], [2, qhi - qlo]])
                nc.scalar.activation(
                    out=sqv[:, qlo:qhi], in_=var_ap,
                    func=mybir.ActivationFunctionType.Sqrt, bias=eps_t[:])
                nc.vector.reciprocal(out=rstd[:, qlo:qhi],
                                     in_=sqv[:, qlo:qhi])
                for q in range(qlo, qhi):
                    eng = nc.gpsimd
                    eng.tensor_scalar(
                        out=Xo[:, q, :], in0=X[:, q, :],
                        scalar1=mv[:, q, 0:1], scalar2=rstd[:, q:q + 1],
                        op0=mybir.AluOpType.subtract,
                        op1=mybir.AluOpType.mult)
                if flags["has_gb"]:
                    nn = qhi - qlo
                    gam_b = bass.AP(
                        tensor=gamma.tensor, offset=gamma[:].offset,
                        ap=[gamma[:].ap[0], [0, nn], [1, D]])
                    bet_b = bass.AP(
                        tensor=beta.tensor, offset=beta[:].offset,
                        ap=[beta[:].ap[0], [0, nn], [1, D]])
                    Xsl = Xo[:, qlo:qhi, :].rearrange("p a b -> p (a b)")
                    nc.vector.tensor_tensor(
                        out=Xsl, in0=Xsl, in1=gam_b, op=mybir.AluOpType.mult)
                    nc.vector.tensor_tensor(
                        out=Xsl, in0=Xsl, in1=bet_b, op=mybir.AluOpType.add)
                nc.sync.dma_start(out=t_out[:, qlo:qhi, :],
                                  in_=Xo[:, qlo:qhi, :])

            # batch index at which each group's chunks complete
            gend = {}
            for i, (c0, cn, g) in enumerate(batches):
                gend[g] = i
            due_b = {gend[g] + SB: g for g in range(ngroups)}
            due_c = {gend[g] + SC: g for g in range(ngroups)}

            for s in range(nb + LOOK + SC + 1):
                if s < nb:
                    emit_front(s)
                if s >= LOOK and s - LOOK < nb:
                    emit_back(s - LOOK)
                sl = s - LOOK
                if sl in due_b:
                    stage_b(due_b[sl])
                if sl in due_c:
                    stage_c(due_c[sl])
                    if due_c[sl] == SPLITG:
                        flush(0, (SPLITG + 1) * GRP)

            # ---- LN tail: flush remaining slots ----
            flush((SPLITG + 1) * GRP, nslot)
    nc.compile()
    return nc


def kernel(**inputs) -> np.ndarray:
    node_feats = np.asarray(inputs["node_feats"], np.float32)
    neighbor_feats = np.asarray(inputs["neighbor_feats"], np.float32)
    per_core, L, nch = _host_prep(
        node_feats, neighbor_feats,
        inputs["edge_indices"], np.asarray(inputs["edge_weights"], np.float32))
    consts, flags = _const_inputs(
        inputs["W1"], inputs["b1"], inputs["W2"], inputs["b2"],
        inputs["Wg"], inputs["bg"], inputs["Wu1"], inputs["bu1"],
        inputs["Wu2"], inputs["bu2"], inputs["gamma"], inputs["beta"])
    if flags["has_b1"] or flags["has_b2"]:
        raise NotImplementedError("nonzero b1/b2 path not wired")

    key = ("nc5", L, nch, tuple(sorted(flags.items())))
    if key not in _CACHE:
        _CACHE.clear()
        _CACHE[key] = _build_nc(L, nch, flags)
    nc = _CACHE[key]

    in_maps = []
    colmaps = []
    for k in range(NCORES):
        m = dict(consts)
        pc = dict(per_core[k])
        colmaps.append(pc.pop("_colmap"))
        sw = pc.pop("_sumwNM")
        wr = pc.pop("_wrow")
        if flags["has_b2"]:
            pc["sumwNM"] = sw
        m.update(pc)
        in_maps.append(m)
    res = run_bass_kernel_spmd(nc, in_maps, core_ids=list(range(NCORES)))
    out = np.empty((N, D), np.float32)
    for k in range(NCORES):
        y = np.asarray(res.results[k]["out"], dtype=np.float32)
        lo = k * NPC
        hi = min(lo + NPC, N)
        nslot = y.shape[1]
        full = y.transpose(1, 0, 2).reshape(nslot * 128, D)
        out[lo:hi] = full[colmaps[k][: hi - lo]]
    return out
